# revision 3
# baseline (speedup 1.0000x reference)
"""Trainium2 Bass kernel for the e-prop gradient fit (nn_Eprop_fit).

Reference computes (B=4, T=300, N=200, NIN=100, K=10):
    dw_in [NIN,N], dw_rec [N,N], dw_out [N,K]
via eligibility traces et[b,t,i,j] = post_term[b,t,j]*pre[b,t,i], exponential
filters over t, and contractions with learning signals.

Reformulation (validated to ~4e-7 rel err vs the jax reference):
  For causal filter F_d(x)[t] = sum_{s<=t} d^{t-s} x[s] and any L:
      sum_t L[t]*F_d(x)[t] = sum_s x[s]*R_d(L)[s],   R_d = anti-causal filter.
  With G = R_lam(L), M = post_term*(G + REG*error2/(B*T)), Mf = R_d(M),
  e1f = R_d(error1):
      dw_in  = sum_{b,s} x[b,s,:]^T      Mf[b,s,:]
      dw_rec = sum_{b,s} z_prev[b,s,:]^T Mf[b,s,:]   (diagonal zeroed)
      dw_out = sum_{b,s} z[b,s,:]^T      e1f[b,s,:]
  post_term[t,j] = psi[t,j] * [no spike in z[t-4..t-1, j]],
  psi = 0.3*relu(1-|(v-thr)/thr|)/thr.

Sharding: 8 cores = (batch b in 0..3) x (post-half h in 0..1). Each core gets
one batch element and 100 of the 200 post neurons; host sums partials over b
and concatenates over h. Time is REVERSED host-side so the anti-causal
filters become forward scans (tensor_tensor_scan along the free axis).
"""

import numpy as np

import concourse.bass as bass
import concourse.tile as tile
from concourse import bacc, mybir
from concourse.bass_utils import run_bass_kernel_spmd
from concourse.masks import make_identity

# problem constants (hardcoded per harness contract)
B, T, N, NIN, K = 4, 300, 200, 100, 10
NH = N // 2          # post-half per core
Q = 3                # time chunks of 100
TC = T // Q          # 100
THRESHOLD = 0.6
DECAY = 0.8
N_REF = 5
REG = 300.0
N_CORES = 8

F32 = mybir.dt.float32

# packed input widths
FEAT_W = 2 * T + 1          # vT | zT | c2  -> [NH, 601]
LHS_W = NIN + N + NH        # x | z_prev | z_h per time row -> [TC, Q, 400]
SMALL_W = T + NH            # e1T | w_outT -> [K, 400]
OUT_W = NIN + N + K         # dwin | dwrec(2x100) | dwout -> [NH, 310]


def _build_program():
    nc = bacc.Bacc("TRN2", target_bir_lowering=False, debug=False,
                   num_devices=N_CORES)

    feat_d = nc.dram_tensor("feat", [NH, FEAT_W], F32, kind="ExternalInput")
    lhs_d = nc.dram_tensor("lhsT", [TC, Q, LHS_W], F32, kind="ExternalInput")
    small_d = nc.dram_tensor("small", [K, SMALL_W], F32, kind="ExternalInput")
    out_d = nc.dram_tensor("out", [NH, OUT_W], F32, kind="ExternalOutput")

    with tile.TileContext(nc) as tc:
        with (
            tc.tile_pool(name="const", bufs=1) as const,
            tc.tile_pool(name="work", bufs=1) as work,
            tc.tile_pool(name="psA", bufs=2, space="PSUM") as psA,
            tc.tile_pool(name="psAcc", bufs=1, space="PSUM") as psAcc,
        ):
            # ---- inputs ----
            feat = work.tile([NH, FEAT_W], F32, tag="feat")
            nc.sync.dma_start(out=feat[:], in_=feat_d.ap())
            lhs = work.tile([TC, Q, LHS_W], F32, tag="lhs")
            nc.sync.dma_start(out=lhs[:], in_=lhs_d.ap())
            small = work.tile([K, SMALL_W], F32, tag="small")
            nc.sync.dma_start(out=small[:], in_=small_d.ap())

            vT = feat[:, 0:T]
            zT = feat[:, T:2 * T]
            c2 = feat[:, 2 * T:2 * T + 1]
            e1T = small[:, 0:T]
            whT = small[:, T:T + NH]

            # ---- constants ----
            dk = const.tile([NH, T], F32, tag="dk")
            nc.vector.memset(dk[:], DECAY)
            ident = const.tile([NH, NH], F32, tag="ident")
            make_identity(nc, ident[:])
            bm1 = const.tile([NH, 1], F32, tag="bm1")
            nc.vector.memset(bm1[:], -1.0)

            # ---- learning signals L[j,u] = sum_k w_out[j,k] e1[u,k] ----
            L_ps = psA.tile([NH, T], F32, tag="psA")
            nc.tensor.matmul(L_ps[:], whT, e1T, start=True, stop=True)

            # G = forward scan (reversed time) of L: g = 0.8*g + L[u]
            G = work.tile([NH, T], F32, tag="G")
            nc.vector.tensor_tensor_scan(
                out=G[:], data0=dk[:], data1=L_ps[:], initial=0.0,
                op0=mybir.AluOpType.mult, op1=mybir.AluOpType.add)

            # ---- psi (without the 0.5 factor; host rescales) ----
            psi_a = work.tile([NH, T], F32, tag="psi_a")
            nc.scalar.activation(psi_a[:], vT,
                                 mybir.ActivationFunctionType.Abs,
                                 bias=bm1[:], scale=1.0 / THRESHOLD)
            psi = work.tile([NH, T], F32, tag="psi")
            nc.scalar.activation(psi[:], psi_a[:],
                                 mybir.ActivationFunctionType.Relu,
                                 bias=1.0, scale=-1.0)

            # ---- refractory mask from spikes in the next 4 (reversed) steps
            zp = work.tile([NH, T + N_REF - 1], F32, tag="zp")
            nc.vector.memset(zp[:, T:T + N_REF - 1], 0.0)
            nc.vector.tensor_scalar(zp[:, 0:T], zT, 0.0, None,
                                    mybir.AluOpType.is_gt)
            r1 = work.tile([NH, T], F32, tag="r1")
            nc.vector.tensor_add(r1[:], zp[:, 1:T + 1], zp[:, 2:T + 2])
            r2 = work.tile([NH, T], F32, tag="r2")
            nc.vector.tensor_add(r2[:], zp[:, 3:T + 3], zp[:, 4:T + 4])
            r = work.tile([NH, T], F32, tag="r")
            nc.vector.tensor_add(r[:], r1[:], r2[:])
            # post = (r == 0) * psi
            post = work.tile([NH, T], F32, tag="post")
            nc.vector.scalar_tensor_tensor(
                out=post[:], in0=r[:], scalar=0.0, in1=psi[:],
                op0=mybir.AluOpType.is_equal, op1=mybir.AluOpType.mult)

            # ---- M = post * (G + c2);  Mf = scan(M) ----
            M = work.tile([NH, T], F32, tag="M")
            nc.vector.scalar_tensor_tensor(
                out=M[:], in0=G[:], scalar=c2, in1=post[:],
                op0=mybir.AluOpType.add, op1=mybir.AluOpType.mult)
            Mf = work.tile([NH, T], F32, tag="Mf")
            nc.vector.tensor_tensor_scan(
                out=Mf[:], data0=dk[:], data1=M[:], initial=0.0,
                op0=mybir.AluOpType.mult, op1=mybir.AluOpType.add)

            # e1f = scan of error1 (rows = K)
            e1f = work.tile([K, T], F32, tag="e1f")
            nc.vector.tensor_tensor_scan(
                out=e1f[:], data0=dk[:K, :], data1=e1T, initial=0.0,
                op0=mybir.AluOpType.mult, op1=mybir.AluOpType.add)

            # ---- transpose Mf, e1f to time-major via PE ----
            MfT = work.tile([TC, Q, NH], F32, tag="MfT")
            e1fT = work.tile([TC, Q, K], F32, tag="e1fT")
            for q in range(Q):
                tr = psA.tile([TC, NH], F32, tag="psA")
                nc.tensor.transpose(tr[:], Mf[:, q * TC:(q + 1) * TC], ident[:])
                nc.scalar.copy(MfT[:, q, :], tr[:])
                trE = psA.tile([TC, K], F32, tag="psA")
                nc.tensor.transpose(trE[:], e1f[:, q * TC:(q + 1) * TC],
                                    ident[:K, :K])
                nc.vector.tensor_copy(e1fT[:, q, :], trE[:])

            # ---- weight-gradient contractions over time chunks ----
            dwin_ps = psAcc.tile([NIN, NH], F32, tag="dwin")
            dwr0_ps = psAcc.tile([NH, NH], F32, tag="dwr0")
            dwr1_ps = psAcc.tile([NH, NH], F32, tag="dwr1")
            dwout_ps = psAcc.tile([NH, K], F32, tag="dwout")
            for q in range(Q):
                st, sp = q == 0, q == Q - 1
                nc.tensor.matmul(dwin_ps[:], lhs[:, q, 0:NIN],
                                 MfT[:, q, :], start=st, stop=sp)
                nc.tensor.matmul(dwr0_ps[:], lhs[:, q, NIN:NIN + NH],
                                 MfT[:, q, :], start=st, stop=sp)
                nc.tensor.matmul(dwr1_ps[:], lhs[:, q, NIN + NH:NIN + N],
                                 MfT[:, q, :], start=st, stop=sp)
                nc.tensor.matmul(dwout_ps[:], lhs[:, q, NIN + N:LHS_W],
                                 e1fT[:, q, :], start=st, stop=sp)

            # ---- pack outputs and store ----
            outt = work.tile([NH, OUT_W], F32, tag="outt")
            nc.scalar.copy(outt[:, 0:NIN], dwin_ps[:])
            nc.vector.tensor_copy(outt[:, NIN:NIN + NH], dwr0_ps[:])
            nc.scalar.copy(outt[:, NIN + NH:NIN + N], dwr1_ps[:])
            nc.vector.tensor_copy(outt[:, NIN + N:OUT_W], dwout_ps[:])
            nc.sync.dma_start(out=out_d.ap(), in_=outt[:])

    nc.compile()
    return nc


_NC_CACHE = None


def _get_nc():
    global _NC_CACHE
    if _NC_CACHE is None:
        _NC_CACHE = _build_program()
    return _NC_CACHE


def _prep_core_inputs(v, z, x, error1, error2, w_out, b, h):
    jsl = slice(h * NH, (h + 1) * NH)
    rev = slice(None, None, -1)

    v_r = v[b, rev, jsl]                       # [T, NH]
    z_r = z[b, rev, :]                         # [T, N]
    x_r = x[b, rev, :]                         # [T, NIN]
    e1_r = error1[b, rev, :]                   # [T, K]
    z_prev = np.concatenate([np.zeros((1, N), np.float32), z[b, :-1, :]], axis=0)
    zprev_r = z_prev[rev, :]                   # [T, N]

    feat = np.empty((NH, FEAT_W), np.float32)
    feat[:, 0:T] = v_r.T
    feat[:, T:2 * T] = z_r[:, jsl].T
    feat[:, 2 * T] = (REG / (B * T)) * error2[jsl]

    lhs = np.empty((TC, Q, LHS_W), np.float32)
    for q in range(Q):
        rows = slice(q * TC, (q + 1) * TC)
        lhs[:, q, 0:NIN] = x_r[rows]
        lhs[:, q, NIN:NIN + N] = zprev_r[rows]
        lhs[:, q, NIN + N:LHS_W] = z_r[rows][:, jsl]

    small = np.zeros((K, SMALL_W), np.float32)
    small[:, 0:T] = e1_r.T
    small[:, T:T + NH] = w_out[jsl, :].T

    return {"feat": feat, "lhsT": np.ascontiguousarray(lhs),
            "small": small}


def kernel(v, z, x, error1, error2, w_out, _trace=False):
    v = np.asarray(v, np.float32)
    z = np.asarray(z, np.float32)
    x = np.asarray(x, np.float32)
    error1 = np.asarray(error1, np.float32)
    error2 = np.asarray(error2, np.float32)
    w_out = np.asarray(w_out, np.float32)

    nc = _get_nc()
    in_maps = [_prep_core_inputs(v, z, x, error1, error2, w_out, c // 2, c % 2)
               for c in range(N_CORES)]
    res = run_bass_kernel_spmd(nc, in_maps, core_ids=list(range(N_CORES)),
                               trace=_trace)

    dw_in = np.zeros((NIN, N), np.float32)
    dw_rec = np.zeros((N, N), np.float32)
    dw_out = np.zeros((N, K), np.float32)
    for h in range(2):
        jsl = slice(h * NH, (h + 1) * NH)
        s = np.zeros((NH, OUT_W), np.float64)
        for b in range(B):
            s += res.results[2 * b + h]["out"]
        dw_in[:, jsl] = 0.5 * s[:, 0:NIN]
        dw_rec[0:NH, jsl] = 0.5 * s[:, NIN:NIN + NH]
        dw_rec[NH:N, jsl] = 0.5 * s[:, NIN + NH:NIN + N]
        dw_out[jsl, :] = s[:, NIN + N:OUT_W]
    np.fill_diagonal(dw_rec, 0.0)

    if _trace:
        return (dw_in, dw_rec, dw_out), res
    return dw_in, dw_rec, dw_out


# revision 6
# speedup vs baseline: 1.1019x; 1.1019x over previous
"""Trainium2 Bass kernel for the e-prop gradient fit (nn_Eprop_fit).

Reference computes (B=4, T=300, N=200, NIN=100, K=10):
    dw_in [NIN,N], dw_rec [N,N], dw_out [N,K]
via eligibility traces et[b,t,i,j] = post_term[b,t,j]*pre[b,t,i], exponential
filters over t, and contractions with learning signals.

Reformulation (validated to ~4e-7 rel err vs the jax reference):
  For causal filter F_d(x)[t] = sum_{s<=t} d^{t-s} x[s] and any L:
      sum_t L[t]*F_d(x)[t] = sum_s x[s]*R_d(L)[s],   R_d = anti-causal filter.
  With G = R_lam(L), M = post_term*(G + REG*error2/(B*T)), Mf = R_d(M),
  e1f = R_d(error1):
      dw_in  = sum_{b,s} x[b,s,:]^T      Mf[b,s,:]
      dw_rec = sum_{b,s} z_prev[b,s,:]^T Mf[b,s,:]   (diagonal zeroed)
      dw_out = sum_{b,s} z[b,s,:]^T      e1f[b,s,:]
  post_term[t,j] = psi[t,j] * [no spike in z[t-4..t-1, j]],
  psi = 0.3*relu(1-|(v-thr)/thr|)/thr.

Sharding: 8 cores = (batch b in 0..3) x (post-half h in 0..1). Each core gets
one batch element and 100 of the 200 post neurons; host sums partials over b
and concatenates over h. Time is REVERSED host-side so the anti-causal
filters become forward scans (tensor_tensor_scan along the free axis).
The z_prev shift is absorbed by transposing overlapping 101-wide chunks of a
zero-prefixed Mf, so z is loaded once (own half first so the one SPMD
program works for both half assignments; host reorders rows on gather).
"""

import numpy as np

import concourse.bass as bass
import concourse.tile as tile
from concourse import bacc, mybir
from concourse.bass_utils import run_bass_kernel_spmd
from concourse.masks import make_identity

# problem constants (hardcoded per harness contract)
B, T, N, NIN, K = 4, 300, 200, 100, 10
NH = N // 2          # post-half per core
Q = 3                # time chunks
TC = T // Q          # 100
THRESHOLD = 0.6
DECAY = 0.8
N_REF = 5
REG = 300.0
N_CORES = 8

F32 = mybir.dt.float32
Alu = mybir.AluOpType
Act = mybir.ActivationFunctionType

# packed input layout
FEAT_W = 2 * T + N_REF              # vT | zT | zero pad(4) | c2 -> [NH, 605]
LHS_W = NIN + N                     # x | z_own | z_other -> [TC, Q, 300]
SMALL_W = T + NH                    # e1T | w_outT -> [K, 400]
OUT_W = NIN + N + K                 # dwin | dwrecA | dwrecB | dwout
N_WARM_PRE = 10                     # PE warm-up matmuls before L
N_WARM_POST = 8                     # and after L, bridging the scan window


def _build_program():
    nc = bacc.Bacc("TRN2", target_bir_lowering=False, debug=False,
                   num_devices=N_CORES)

    feat_d = nc.dram_tensor("feat", [NH, FEAT_W], F32, kind="ExternalInput")
    lhs_d = nc.dram_tensor("lhsT", [TC, Q, LHS_W], F32, kind="ExternalInput")
    small_d = nc.dram_tensor("small", [K, SMALL_W], F32, kind="ExternalInput")
    out_d = nc.dram_tensor("out", [NH, OUT_W], F32, kind="ExternalOutput")

    with tile.TileContext(nc) as tc:
        with (
            tc.tile_pool(name="const", bufs=1) as const,
            tc.tile_pool(name="work", bufs=1) as work,
            tc.tile_pool(name="psA", bufs=3, space="PSUM") as psA,
            tc.tile_pool(name="psW", bufs=1, space="PSUM") as psW,
            tc.tile_pool(name="psAcc", bufs=1, space="PSUM") as psAcc,
        ):
            # ---- inputs: three parallel DMA paths ----
            small = work.tile([K, SMALL_W], F32, tag="small")
            lhs = work.tile([TC, Q, LHS_W], F32, tag="lhs")
            feat = work.tile([NH, FEAT_W], F32, tag="feat")
            nc.gpsimd.dma_start(out=small[:], in_=small_d.ap())
            nc.gpsimd.dma_start(out=lhs[:], in_=lhs_d.ap())
            nc.sync.dma_start(out=feat[:, 0:T], in_=feat_d.ap()[:, 0:T])
            nc.scalar.dma_start(out=feat[:, T:FEAT_W],
                                in_=feat_d.ap()[:, T:FEAT_W])

            vT = feat[:, 0:T]
            c2 = feat[:, FEAT_W - 1:FEAT_W]
            e1T = small[:, 0:T]
            whT = small[:, T:T + NH]

            # ---- constants ----
            dk = const.tile([NH, T], F32, tag="dk")
            nc.vector.memset(dk[:], DECAY)
            bm1 = const.tile([NH, 1], F32, tag="bm1")
            nc.vector.memset(bm1[:], -1.0)
            scr = const.tile([NH, 1], F32, tag="scr")
            ident = const.tile([NH, NH], F32, tag="ident")
            make_identity(nc, ident[:])

            # PE warm-up: contentless matmuls so HAM un-throttles before the
            # real transpose/contraction burst. Inputs: dk only.
            warm_ps = psW.tile([NH, 64], F32, tag="warm")
            for _ in range(N_WARM_PRE):
                nc.tensor.matmul(warm_ps[:], dk[:, 0:NH], dk[:, 0:64],
                                 start=True, stop=True)

            # early dummy activation so the ACT table loads during DMA wait
            nc.scalar.activation(scr[:], bm1[:], Act.Abs)

            # ---- learning signals L[j,u] = sum_k w_out[j,k] e1[u,k] ----
            L_ps = psA.tile([NH, T], F32, tag="psA")
            nc.tensor.matmul(L_ps[:], whT, e1T, start=True, stop=True)

            for _ in range(N_WARM_POST):
                nc.tensor.matmul(warm_ps[:], dk[:, 0:NH], dk[:, 0:64],
                                 start=True, stop=True)

            # G = forward scan (in reversed time) of L: g = 0.8*g + L[u]
            G = work.tile([NH, T], F32, tag="G")
            nc.vector.tensor_tensor_scan(
                out=G[:], data0=dk[:], data1=L_ps[:], initial=0.0,
                op0=Alu.mult, op1=Alu.add)

            # ---- psi (without the 0.5 factor; host rescales) ----
            psi_a = work.tile([NH, T], F32, tag="psi_a")
            nc.scalar.activation(psi_a[:], vT, Act.Abs,
                                 bias=bm1[:], scale=1.0 / THRESHOLD)
            psi = work.tile([NH, T], F32, tag="psi")
            nc.scalar.activation(psi[:], psi_a[:], Act.Relu,
                                 bias=1.0, scale=-1.0)

            # ---- refractory: r[j,u] = sum_{w=1..4} z[j,u+w] (z >= 0, padded)
            fz = feat[:, T + 1:T + 1 + T]
            cstride = fz.ap[1][0]
            win = bass.AP(tensor=fz.tensor, offset=fz.offset,
                          ap=[fz.ap[0], [cstride, T], [cstride, N_REF - 1]])
            r = work.tile([NH, T], F32, tag="r")
            nc.vector.tensor_reduce(r[:], win, mybir.AxisListType.X, Alu.add)
            # post = (r == 0) * psi
            post = work.tile([NH, T], F32, tag="post")
            nc.vector.scalar_tensor_tensor(
                out=post[:], in0=r[:], scalar=0.0, in1=psi[:],
                op0=Alu.is_equal, op1=Alu.mult)

            # ---- M = post * (G + c2);  Mf = zero-prefixed scan of M ----
            M = work.tile([NH, T], F32, tag="M")
            nc.vector.scalar_tensor_tensor(
                out=M[:], in0=G[:], scalar=c2, in1=post[:],
                op0=Alu.add, op1=Alu.mult)
            Mft = work.tile([NH, T + 1], F32, tag="Mft")
            nc.vector.memset(Mft[:, 0:1], 0.0)
            nc.vector.tensor_tensor_scan(
                out=Mft[:, 1:T + 1], data0=dk[:], data1=M[:], initial=0.0,
                op0=Alu.mult, op1=Alu.add)

            # e1f = scan of error1
            e1f = work.tile([K, T], F32, tag="e1f")
            nc.vector.tensor_tensor_scan(
                out=e1f[:], data0=dk[:K, :], data1=e1T, initial=0.0,
                op0=Alu.mult, op1=Alu.add)

            # ---- transpose Mf chunks via PE ----
            # MfT rows = Mf[u] (dw_in rhs); MfS rows = Mf[u-1] (dw_rec rhs,
            # absorbing the z_prev time shift via the zero-prefixed Mft).
            MfT = work.tile([TC, Q, NH], F32, tag="MfT")
            MfS = work.tile([TC, Q, NH], F32, tag="MfS")
            e1fT = work.tile([TC, Q, K], F32, tag="e1fT")
            for q in range(Q):
                trT = psA.tile([TC, NH], F32, tag="psA")
                nc.tensor.transpose(trT[:], Mft[:, q * TC + 1:q * TC + TC + 1],
                                    ident[:])
                nc.scalar.copy(MfT[:, q, :], trT[:])
                trS = psA.tile([TC, NH], F32, tag="psA")
                nc.tensor.transpose(trS[:], Mft[:, q * TC:q * TC + TC],
                                    ident[:])
                nc.vector.tensor_copy(MfS[:, q, :], trS[:])
            for q in range(Q):
                trE = psA.tile([TC, K], F32, tag="psA")
                nc.tensor.transpose(trE[:], e1f[:, q * TC:(q + 1) * TC],
                                    ident[:K, :K])
                nc.vector.tensor_copy(e1fT[:, q, :], trE[:])

            # ---- weight-gradient contractions over time chunks ----
            dwin_ps = psAcc.tile([NIN, NH], F32, tag="dwin")
            dwrA_ps = psAcc.tile([NH, NH], F32, tag="dwrA")
            dwrB_ps = psAcc.tile([NH, NH], F32, tag="dwrB")
            dwout_ps = psAcc.tile([NH, K], F32, tag="dwout")
            for q in range(Q):
                st, sp = q == 0, q == Q - 1
                nc.tensor.matmul(dwin_ps[:], lhs[:, q, 0:NIN], MfT[:, q, :],
                                 start=st, stop=sp)
                nc.tensor.matmul(dwrA_ps[:], lhs[:, q, NIN:NIN + NH],
                                 MfS[:, q, :], start=st, stop=sp)
                nc.tensor.matmul(dwrB_ps[:], lhs[:, q, NIN + NH:LHS_W],
                                 MfS[:, q, :], start=st, stop=sp)
                nc.tensor.matmul(dwout_ps[:], lhs[:, q, NIN:NIN + NH],
                                 e1fT[:, q, :], start=st, stop=sp)

            # ---- pack outputs and store ----
            outt = work.tile([NH, OUT_W], F32, tag="outt")
            nc.scalar.copy(outt[:, 0:NIN], dwin_ps[:])
            nc.scalar.copy(outt[:, NIN:NIN + NH], dwrA_ps[:])
            nc.vector.tensor_copy(outt[:, NIN + NH:NIN + N], dwrB_ps[:])
            nc.vector.tensor_copy(outt[:, NIN + N:OUT_W], dwout_ps[:])
            nc.sync.dma_start(out=out_d.ap(), in_=outt[:])

    nc.compile()
    return nc


_NC_CACHE = None


def _get_nc():
    global _NC_CACHE
    if _NC_CACHE is None:
        _NC_CACHE = _build_program()
    return _NC_CACHE


def _prep_core_inputs(v, z, x, error1, error2, w_out, b, h):
    jsl = slice(h * NH, (h + 1) * NH)
    osl = slice((1 - h) * NH, (2 - h) * NH)
    rev = slice(None, None, -1)

    v_r = v[b, rev, jsl]                       # [T, NH]
    z_r = z[b, rev, :]                         # [T, N]
    x_r = x[b, rev, :]                         # [T, NIN]

    feat = np.zeros((NH, FEAT_W), np.float32)
    feat[:, 0:T] = v_r.T
    feat[:, T:2 * T] = z_r[:, jsl].T           # cols 600:604 stay zero
    feat[:, FEAT_W - 1] = (REG / (B * T)) * error2[jsl]

    lhs = np.empty((TC, Q, LHS_W), np.float32)
    for q in range(Q):
        rows = slice(q * TC, (q + 1) * TC)
        lhs[:, q, 0:NIN] = x_r[rows]
        lhs[:, q, NIN:NIN + NH] = z_r[rows][:, jsl]
        lhs[:, q, NIN + NH:LHS_W] = z_r[rows][:, osl]

    small = np.zeros((K, SMALL_W), np.float32)
    small[:, 0:T] = error1[b, rev, :].T
    small[:, T:T + NH] = w_out[jsl, :].T

    return {"feat": feat, "lhsT": np.ascontiguousarray(lhs), "small": small}


def kernel(v, z, x, error1, error2, w_out, _trace=False):
    v = np.asarray(v, np.float32)
    z = np.asarray(z, np.float32)
    x = np.asarray(x, np.float32)
    error1 = np.asarray(error1, np.float32)
    error2 = np.asarray(error2, np.float32)
    w_out = np.asarray(w_out, np.float32)

    nc = _get_nc()
    in_maps = [_prep_core_inputs(v, z, x, error1, error2, w_out, c // 2, c % 2)
               for c in range(N_CORES)]
    res = run_bass_kernel_spmd(nc, in_maps, core_ids=list(range(N_CORES)),
                               trace=_trace)

    dw_in = np.zeros((NIN, N), np.float32)
    dw_rec = np.zeros((N, N), np.float32)
    dw_out = np.zeros((N, K), np.float32)
    for h in range(2):
        jsl = slice(h * NH, (h + 1) * NH)
        osl = slice((1 - h) * NH, (2 - h) * NH)
        s = np.zeros((NH, OUT_W), np.float64)
        for b in range(B):
            s += res.results[2 * b + h]["out"]
        dw_in[:, jsl] = 0.5 * s[:, 0:NIN]
        dw_rec[jsl, jsl] = 0.5 * s[:, NIN:NIN + NH]      # own-half rows
        dw_rec[osl, jsl] = 0.5 * s[:, NIN + NH:NIN + N]  # other-half rows
        dw_out[jsl, :] = s[:, NIN + N:OUT_W]
    np.fill_diagonal(dw_rec, 0.0)

    if _trace:
        return (dw_in, dw_rec, dw_out), res
    return dw_in, dw_rec, dw_out


# revision 8
# speedup vs baseline: 1.1706x; 1.0623x over previous
"""Trainium2 Bass kernel for the e-prop gradient fit (nn_Eprop_fit).

Reference computes (B=4, T=300, N=200, NIN=100, K=10):
    dw_in [NIN,N], dw_rec [N,N], dw_out [N,K]
via eligibility traces et[b,t,i,j] = post_term[b,t,j]*pre[b,t,i], exponential
filters over t, and contractions with learning signals.

Reformulation (validated to ~4e-7 rel err vs the jax reference):
  For causal filter F_d(x)[t] = sum_{s<=t} d^{t-s} x[s] and any L:
      sum_t L[t]*F_d(x)[t] = sum_s x[s]*R_d(L)[s],   R_d = anti-causal filter.
  With G = R_lam(L), M = post_term*(G + REG*error2/(B*T)), Mf = R_d(M),
  e1f = R_d(error1):
      dw_in  = sum_{b,s} x[b,s,:]^T      Mf[b,s,:]
      dw_rec = sum_{b,s} z_prev[b,s,:]^T Mf[b,s,:]   (diagonal zeroed)
      dw_out = sum_{b,s} z[b,s,:]^T      e1f[b,s,:]
  post_term[t,j] = psi[t,j] * [no spike in z[t-4..t-1, j]],
  psi = 0.3*relu(1-|(v-thr)/thr|)/thr.

Sharding: 8 cores = (batch b in 0..3) x (post-half h in 0..1). Each core gets
one batch element and 100 of the 200 post neurons; host sums partials over b
and concatenates over h (own half packed first so one SPMD program serves
both halves). Time is REVERSED host-side so the anti-causal filters become
forward scans (tensor_tensor_scan along the free axis). The z_prev shift is
applied host-side to z (zsh[u] = z_u[u+1]); with e1f likewise read shifted,
dw_rec(own) and dw_out share lhsT and fuse into one N=110 matmul, plus a
rank-1 correction for the dropped u=0 term of dw_out.
"""

import numpy as np

import concourse.bass as bass
import concourse.tile as tile
from concourse import bacc, mybir
from concourse.bass_utils import run_bass_kernel_spmd
from concourse.masks import make_identity

# problem constants (hardcoded per harness contract)
B, T, N, NIN, K = 4, 300, 200, 100, 10
NH = N // 2          # post-half per core
Q = 3                # time chunks
TC = T // Q          # 100
THRESHOLD = 0.6
DECAY = 0.8
N_REF = 5
REG = 300.0
N_CORES = 8

F32 = mybir.dt.float32
Alu = mybir.AluOpType
Act = mybir.ActivationFunctionType

# packed input layout
FEAT_W = 2 * T + N_REF              # vT | zT | zero pad(4) | c2 -> [NH, 605]
LHS_W = NIN + N                     # x | zsh_own | zsh_other -> [TC, Q, 300]
SMALL_W = T + 2 * NH + K            # e1T | w_outT | zrow | e1row -> [K, 510]
OUT_W = NIN + N + K                 # dwin | dwrecA | dwrecB | dwout


def _build_program():
    nc = bacc.Bacc("TRN2", target_bir_lowering=False, debug=False,
                   num_devices=N_CORES)

    feat_d = nc.dram_tensor("feat", [NH, FEAT_W], F32, kind="ExternalInput")
    lhs_d = nc.dram_tensor("lhsT", [TC, Q, LHS_W], F32, kind="ExternalInput")
    small_d = nc.dram_tensor("small", [K, SMALL_W], F32, kind="ExternalInput")
    out_d = nc.dram_tensor("out", [NH, OUT_W], F32, kind="ExternalOutput")

    with tile.TileContext(nc) as tc:
        with (
            tc.tile_pool(name="const", bufs=1) as const,
            tc.tile_pool(name="work", bufs=1) as work,
            tc.tile_pool(name="psA", bufs=4, space="PSUM") as psA,
            tc.tile_pool(name="psAcc", bufs=1, space="PSUM") as psAcc,
        ):
            # ---- inputs: HWDGE queues; small first (it gates L -> G) ----
            small = work.tile([K, SMALL_W], F32, tag="small")
            lhs = work.tile([TC, Q, LHS_W], F32, tag="lhs")
            feat = work.tile([NH, FEAT_W], F32, tag="feat")
            nc.sync.dma_start(out=small[:], in_=small_d.ap())
            nc.scalar.dma_start(out=feat[:, T:FEAT_W],
                                in_=feat_d.ap()[:, T:FEAT_W])
            nc.sync.dma_start(out=feat[:, 0:T], in_=feat_d.ap()[:, 0:T])
            nc.sync.dma_start(out=lhs[:], in_=lhs_d.ap())

            vT = feat[:, 0:T]
            c2 = feat[:, FEAT_W - 1:FEAT_W]
            e1T = small[:, 0:T]
            whT = small[:, T:T + NH]

            # ---- constants ----
            dk = const.tile([NH, T], F32, tag="dk")
            nc.vector.memset(dk[:], DECAY)
            bm1 = const.tile([NH, 1], F32, tag="bm1")
            nc.vector.memset(bm1[:], -1.0)
            scr = const.tile([NH, 1], F32, tag="scr")
            ident = const.tile([NH, NH], F32, tag="ident")
            make_identity(nc, ident[:])

            # early dummy activation so the ACT table loads during DMA wait
            nc.scalar.activation(scr[:], bm1[:], Act.Abs)

            # ---- learning signals L[j,u] = sum_k w_out[j,k] e1[u,k] ----
            L_ps = psA.tile([NH, T], F32, tag="psA")
            nc.tensor.matmul(L_ps[:], whT, e1T, start=True, stop=True)

            # e1f scan (early: only needs small); col T stays zero so the
            # shifted read e1fS[u] = e1f[u+1] is zero-padded at u = T-1.
            e1fp = work.tile([K, T + 1], F32, tag="e1fp")
            nc.vector.memset(e1fp[:, T:T + 1], 0.0)
            nc.vector.tensor_tensor_scan(
                out=e1fp[:, 0:T], data0=dk[:K, :], data1=e1T, initial=0.0,
                op0=Alu.mult, op1=Alu.add)

            # ---- refractory: r[j,u] = sum_{w=1..4} z[j,u+w] (z>=0, padded)
            fz = feat[:, T + 1:T + 1 + T]
            cstride = fz.ap[1][0]
            win = bass.AP(tensor=fz.tensor, offset=fz.offset,
                          ap=[fz.ap[0], [cstride, T], [cstride, N_REF - 1]])
            r = work.tile([NH, T], F32, tag="r")
            nc.vector.tensor_reduce(r[:], win, mybir.AxisListType.X, Alu.add)

            # G = forward scan (in reversed time) of L: g = 0.8*g + L[u]
            G = work.tile([NH, T], F32, tag="G")
            nc.vector.tensor_tensor_scan(
                out=G[:], data0=dk[:], data1=L_ps[:], initial=0.0,
                op0=Alu.mult, op1=Alu.add)

            # ---- psi (without the 0.5 factor; host rescales) ----
            psi_a = work.tile([NH, T], F32, tag="psi_a")
            nc.scalar.activation(psi_a[:], vT, Act.Abs,
                                 bias=bm1[:], scale=1.0 / THRESHOLD)
            psi = work.tile([NH, T], F32, tag="psi")
            nc.scalar.activation(psi[:], psi_a[:], Act.Relu,
                                 bias=1.0, scale=-1.0)

            # post = (r == 0) * psi;  M = post * (G + c2);  Mf = scan(M)
            post = work.tile([NH, T], F32, tag="post")
            nc.vector.scalar_tensor_tensor(
                out=post[:], in0=r[:], scalar=0.0, in1=psi[:],
                op0=Alu.is_equal, op1=Alu.mult)
            M = work.tile([NH, T], F32, tag="M")
            nc.vector.scalar_tensor_tensor(
                out=M[:], in0=G[:], scalar=c2, in1=post[:],
                op0=Alu.add, op1=Alu.mult)
            Mf = work.tile([NH, T], F32, tag="Mf")
            nc.vector.tensor_tensor_scan(
                out=Mf[:], data0=dk[:], data1=M[:], initial=0.0,
                op0=Alu.mult, op1=Alu.add)

            # ---- transposes to time-major; rhsP = [MfT | e1fS] ----
            rhsP = work.tile([TC, Q, NH + K], F32, tag="rhsP")
            for q in range(Q):
                trE = psA.tile([TC, K], F32, tag="psA")
                nc.tensor.transpose(
                    trE[:], e1fp[:, q * TC + 1:(q + 1) * TC + 1],
                    ident[:K, :K])
                nc.vector.tensor_copy(rhsP[:, q, NH:NH + K], trE[:])
            for q in range(Q):
                trT = psA.tile([TC, NH], F32, tag="psA")
                nc.tensor.transpose(trT[:], Mf[:, q * TC:(q + 1) * TC],
                                    ident[:])
                nc.scalar.copy(rhsP[:, q, 0:NH], trT[:])

            # ---- weight-gradient contractions over time chunks ----
            dwin_ps = psAcc.tile([NIN, NH], F32, tag="dwin")
            dwA_ps = psAcc.tile([NH, NH + K], F32, tag="dwA")
            dwB_ps = psAcc.tile([NH, NH], F32, tag="dwB")
            for q in range(Q):
                st, sp = q == 0, q == Q - 1
                nc.tensor.matmul(dwin_ps[:], lhs[:, q, 0:NIN],
                                 rhsP[:, q, 0:NH], start=st, stop=sp)
                nc.tensor.matmul(dwA_ps[:], lhs[:, q, NIN:NIN + NH],
                                 rhsP[:, q, :], start=st, stop=False)
                nc.tensor.matmul(dwB_ps[:], lhs[:, q, NIN + NH:LHS_W],
                                 rhsP[:, q, 0:NH], start=st, stop=sp)
            # rank-1 correction: dw_out += z_u[0] (x) e1f[0]
            nc.tensor.matmul(dwA_ps[:, NH:NH + K], small[0:1, T + NH:T + 2 * NH],
                             small[0:1, T + 2 * NH:T + 2 * NH + K],
                             start=False, stop=True)

            # ---- pack outputs and store ----
            outt = work.tile([NH, OUT_W], F32, tag="outt")
            nc.scalar.copy(outt[:, 0:NIN], dwin_ps[:])
            nc.scalar.copy(outt[:, NIN:NIN + NH], dwA_ps[:, 0:NH])
            nc.vector.tensor_copy(outt[:, NIN + NH:NIN + N], dwB_ps[:])
            nc.vector.tensor_copy(outt[:, NIN + N:OUT_W], dwA_ps[:, NH:NH + K])
            nc.sync.dma_start(out=out_d.ap(), in_=outt[:])

    nc.compile()
    return nc


_NC_CACHE = None


def _get_nc():
    global _NC_CACHE
    if _NC_CACHE is None:
        _NC_CACHE = _build_program()
    return _NC_CACHE


def _prep_core_inputs(v, z, x, error1, error2, w_out, b, h):
    jsl = slice(h * NH, (h + 1) * NH)
    osl = slice((1 - h) * NH, (2 - h) * NH)
    rev = slice(None, None, -1)

    z_r = z[b, rev, :]                         # [T, N]
    x_r = x[b, rev, :]                         # [T, NIN]
    zsh = np.vstack([z_r[1:], np.zeros((1, N), np.float32)])

    feat = np.zeros((NH, FEAT_W), np.float32)
    feat[:, 0:T] = v[b, rev, jsl].T
    feat[:, T:2 * T] = z_r[:, jsl].T           # cols 600:604 stay zero
    feat[:, FEAT_W - 1] = (REG / (B * T)) * error2[jsl]

    lhs = np.empty((TC, Q, LHS_W), np.float32)
    for q in range(Q):
        rows = slice(q * TC, (q + 1) * TC)
        lhs[:, q, 0:NIN] = x_r[rows]
        lhs[:, q, NIN:NIN + NH] = zsh[rows][:, jsl]
        lhs[:, q, NIN + NH:LHS_W] = zsh[rows][:, osl]

    small = np.zeros((K, SMALL_W), np.float32)
    small[:, 0:T] = error1[b, rev, :].T
    small[:, T:T + NH] = w_out[jsl, :].T
    small[0, T + NH:T + 2 * NH] = z_r[0, jsl]      # z_u[0] (own half)
    small[0, T + 2 * NH:T + 2 * NH + K] = error1[b, T - 1, :]  # e1f[0]

    return {"feat": feat, "lhsT": np.ascontiguousarray(lhs), "small": small}


def kernel(v, z, x, error1, error2, w_out, _trace=False):
    v = np.asarray(v, np.float32)
    z = np.asarray(z, np.float32)
    x = np.asarray(x, np.float32)
    error1 = np.asarray(error1, np.float32)
    error2 = np.asarray(error2, np.float32)
    w_out = np.asarray(w_out, np.float32)

    nc = _get_nc()
    in_maps = [_prep_core_inputs(v, z, x, error1, error2, w_out, c // 2, c % 2)
               for c in range(N_CORES)]
    res = run_bass_kernel_spmd(nc, in_maps, core_ids=list(range(N_CORES)),
                               trace=_trace)

    dw_in = np.zeros((NIN, N), np.float32)
    dw_rec = np.zeros((N, N), np.float32)
    dw_out = np.zeros((N, K), np.float32)
    for h in range(2):
        jsl = slice(h * NH, (h + 1) * NH)
        osl = slice((1 - h) * NH, (2 - h) * NH)
        s = np.zeros((NH, OUT_W), np.float64)
        for b in range(B):
            s += res.results[2 * b + h]["out"]
        dw_in[:, jsl] = 0.5 * s[:, 0:NIN]
        dw_rec[jsl, jsl] = 0.5 * s[:, NIN:NIN + NH]      # own-half rows
        dw_rec[osl, jsl] = 0.5 * s[:, NIN + NH:NIN + N]  # other-half rows
        dw_out[jsl, :] = s[:, NIN + N:OUT_W]
    np.fill_diagonal(dw_rec, 0.0)

    if _trace:
        return (dw_in, dw_rec, dw_out), res
    return dw_in, dw_rec, dw_out


# revision 15
# speedup vs baseline: 1.3099x; 1.1190x over previous
"""Trainium2 Bass kernel for the e-prop gradient fit (nn_Eprop_fit).

Reference computes (B=4, T=300, N=200, NIN=100, K=10):
    dw_in [NIN,N], dw_rec [N,N], dw_out [N,K]
via eligibility traces et[b,t,i,j] = post_term[b,t,j]*pre[b,t,i], exponential
filters over t, and contractions with learning signals.

Reformulation (validated to ~4e-7 rel err vs the jax reference):
  For causal filter F_d(x)[t] = sum_{s<=t} d^{t-s} x[s] and any L:
      sum_t L[t]*F_d(x)[t] = sum_s x[s]*R_d(L)[s],   R_d = anti-causal filter.
  With G = R_lam(L), M = post_term*(G + REG*error2/(B*T)), Mf = R_d(M),
  e1f = R_d(error1):
      dw_in  = sum_{b,s} x[b,s,:]^T      Mf[b,s,:]
      dw_rec = sum_{b,s} z_prev[b,s,:]^T Mf[b,s,:]   (diagonal zeroed)
      dw_out = sum_{b,s} z[b,s,:]^T      e1f[b,s,:]
  post_term[t,j] = psi[t,j] * [no spike in z[t-4..t-1, j]],
  psi = 0.3*relu(1-|(v-thr)/thr|)/thr.

Sharding: 8 cores = (batch b in 0..3) x (post-half h in 0..1). Each core gets
one batch element and 100 of the 200 post neurons; host sums partials over b
and concatenates over h (own half packed first so one SPMD program serves
both halves). Time is REVERSED host-side so the anti-causal filters become
forward scans (tensor_tensor_scan along the free axis). The z_prev shift is
applied host-side to z (zsh[u] = z_u[u+1]); with e1f likewise read shifted,
dw_rec(own) and dw_out share lhsT and fuse into one N=110 matmul, plus a
rank-1 correction for the dropped u=0 term of dw_out.
"""

import numpy as np

import concourse.bass as bass
import concourse.tile as tile
from concourse import bacc, mybir
from concourse.bass_utils import run_bass_kernel_spmd

# problem constants (hardcoded per harness contract)
B, T, N, NIN, K = 4, 300, 200, 100, 10
NH = N // 2          # post-half per core
Q = 3                # time chunks
TC = T // Q          # 100
THRESHOLD = 0.6
DECAY = 0.8
N_REF = 5
REG = 300.0
N_CORES = 8

F32 = mybir.dt.float32
F32R = mybir.dt.float32r
USE_F32R = True
MM_DT = F32R if USE_F32R else F32
Alu = mybir.AluOpType
Act = mybir.ActivationFunctionType

# packed input layout
FEAT_W = 2 * T + N_REF              # vT | zT | zero pad(4) | c2 -> [NH, 605]
LHS_W = NIN + N                     # x | zsh_own | zsh_other -> [TC, Q, 300]
SMALL_W = T + 1 + 2 * NH + K        # e1T | 0 | w_outT | zrow | e1row -> [K, 511]
OUT_W = NIN + N + K                 # dwin | dwrecA | dwrecB | dwout


def _mm(ap):
    if USE_F32R and ap.dtype != F32R:
        return ap.bitcast(F32R)
    return ap


def _build_program():
    nc = bacc.Bacc("TRN2", target_bir_lowering=False, debug=False,
                   num_devices=N_CORES)

    feat_d = nc.dram_tensor("feat", [NH, FEAT_W], F32, kind="ExternalInput")
    lhs_d = nc.dram_tensor("lhsT", [TC, Q, LHS_W], MM_DT, kind="ExternalInput")
    small_d = nc.dram_tensor("small", [K, SMALL_W], MM_DT, kind="ExternalInput")
    ident_d = nc.dram_tensor("identd", [NH, NH], F32, kind="ExternalInput")
    out_d = nc.dram_tensor("out", [NH, OUT_W], F32, kind="ExternalOutput")

    with tile.TileContext(nc) as tc:
        with (
            tc.tile_pool(name="const", bufs=1) as const,
            tc.tile_pool(name="work", bufs=1) as work,
            tc.tile_pool(name="psA", bufs=4, space="PSUM") as psA,
            tc.tile_pool(name="psAcc", bufs=1, space="PSUM") as psAcc,
        ):
            # ---- inputs: HWDGE queues; small first (it gates L -> G) ----
            small = work.tile([K, SMALL_W], MM_DT, tag="small")
            lhs = work.tile([TC, Q, LHS_W], MM_DT, tag="lhs")
            feat = work.tile([NH, FEAT_W], F32, tag="feat")
            ident = const.tile([NH, NH], F32, tag="ident")
            nc.sync.dma_start(out=feat[:, T:FEAT_W],
                               in_=feat_d.ap()[:, T:FEAT_W])
            nc.sync.dma_start(out=small[:], in_=small_d.ap())
            nc.scalar.dma_start(out=feat[:, 0:T], in_=feat_d.ap()[:, 0:T])
            nc.sync.dma_start(out=ident[:], in_=ident_d.ap())
            nc.sync.dma_start(out=lhs[:], in_=lhs_d.ap())

            vT = feat[:, 0:T]
            c2 = feat[:, FEAT_W - 1:FEAT_W]
            e1T = small[:, 0:T]
            e1T_f = e1T.bitcast(F32)
            whT = small[:, T + 1:T + 1 + NH]

            # ---- constants ----
            dk = const.tile([NH, T + 1], F32, tag="dk")
            nc.vector.memset(dk[:], DECAY)
            bm1 = const.tile([NH, 1], F32, tag="bm1")
            nc.vector.memset(bm1[:], -1.0)
            scr = const.tile([NH, 1], F32, tag="scr")

            # early dummy activation so the ACT table loads during DMA wait
            nc.scalar.activation(scr[:], bm1[:], Act.Abs)

            # ---- learning signals L[j,u] = sum_k w_out[j,k] e1[u,k] ----
            L_ps = psA.tile([NH, T], F32, tag="psA")
            nc.tensor.matmul(L_ps[:], _mm(whT), _mm(e1T), start=True, stop=True)

            # ---- refractory (first on DVE: feat-z lands earliest): r[j,u] = sum_{w=1..4} z[j,u+w] (z>=0, padded)
            fz = feat[:, T + 1:T + 1 + T]
            cstride = fz.ap[1][0]
            win = bass.AP(tensor=fz.tensor, offset=fz.offset,
                          ap=[fz.ap[0], [cstride, T], [cstride, N_REF - 1]])
            r = work.tile([NH, T], F32, tag="r")
            nc.vector.tensor_reduce(r[:], win, mybir.AxisListType.X, Alu.add)

            # e1f scan (needs only small); col T stays zero so the
            # shifted read e1fS[u] = e1f[u+1] is zero-padded at u = T-1.
            e1fp = work.tile([K, T + 1], F32, tag="e1fp")
            nc.vector.tensor_tensor_scan(
                out=e1fp[:], data0=dk[:K, :],
                data1=small[:, 0:T + 1].bitcast(F32), initial=0.0,
                op0=Alu.mult, op1=Alu.add)

            # G = forward scan (in reversed time) of L: g = 0.8*g + L[u]
            G = work.tile([NH, T], F32, tag="G")
            nc.vector.tensor_tensor_scan(
                out=G[:], data0=dk[:, 0:T], data1=L_ps[:], initial=0.0,
                op0=Alu.mult, op1=Alu.add)

            # ---- psi (without the 0.5 factor; host rescales) ----
            psi_a = work.tile([NH, T], F32, tag="psi_a")
            nc.scalar.activation(psi_a[:], vT, Act.Abs,
                                 bias=bm1[:], scale=1.0 / THRESHOLD)
            psi = work.tile([NH, T], F32, tag="psi")
            nc.scalar.activation(psi[:], psi_a[:], Act.Relu,
                                 bias=1.0, scale=-1.0)

            # post = (r == 0) * psi;  M = post * (G + c2);  Mf = scan(M)
            post = work.tile([NH, T], F32, tag="post")
            nc.vector.scalar_tensor_tensor(
                out=post[:], in0=r[:], scalar=0.0, in1=psi[:],
                op0=Alu.is_equal, op1=Alu.mult)
            M = work.tile([NH, T], F32, tag="M")
            nc.vector.scalar_tensor_tensor(
                out=M[:], in0=G[:], scalar=c2, in1=post[:],
                op0=Alu.add, op1=Alu.mult)
            Mf = work.tile([NH, T], F32, tag="Mf")
            nc.vector.tensor_tensor_scan(
                out=Mf[:], data0=dk[:, 0:T], data1=M[:], initial=0.0,
                op0=Alu.mult, op1=Alu.add)

            # ---- transposes to time-major; rhsP = [MfT | e1fS] ----
            rhsP = work.tile([TC, Q, NH + K], MM_DT, tag="rhsP")
            for q in range(Q):
                trE = psA.tile([TC, K], F32, tag="psA")
                nc.tensor.transpose(
                    trE[:], e1fp[:, q * TC + 1:(q + 1) * TC + 1],
                    ident[:K, :K])
                nc.vector.tensor_copy(rhsP[:, q, NH:NH + K], trE[:])
            for q in range(Q):
                trT = psA.tile([TC, NH], F32, tag="psA")
                nc.tensor.transpose(trT[:], Mf[:, q * TC:(q + 1) * TC],
                                    ident[:])
                nc.scalar.copy(rhsP[:, q, 0:NH], trT[:])

            # ---- weight-gradient contractions over time chunks ----
            dwin_ps = psAcc.tile([NIN, NH], F32, tag="dwin")
            dwA_ps = psAcc.tile([NH, NH + K], F32, tag="dwA")
            dwB_ps = psAcc.tile([NH, NH], F32, tag="dwB")
            for q in range(Q):
                st, sp = q == 0, q == Q - 1
                nc.tensor.matmul(dwin_ps[:], _mm(lhs[:, q, 0:NIN]),
                                 _mm(rhsP[:, q, 0:NH]), start=st, stop=sp)
                nc.tensor.matmul(dwA_ps[:], _mm(lhs[:, q, NIN:NIN + NH]),
                                 _mm(rhsP[:, q, :]), start=st, stop=False)
                nc.tensor.matmul(dwB_ps[:], _mm(lhs[:, q, NIN + NH:LHS_W]),
                                 _mm(rhsP[:, q, 0:NH]), start=st, stop=sp)
            # rank-1 correction: dw_out += z_u[0] (x) e1f[0]
            nc.tensor.matmul(dwA_ps[:, NH:NH + K],
                             _mm(small[0:1, T + 1 + NH:T + 1 + 2 * NH]),
                             _mm(small[0:1, T + 1 + 2 * NH:T + 1 + 2 * NH + K]),
                             start=False, stop=True)

            # ---- pack outputs and store ----
            outt = work.tile([NH, OUT_W], F32, tag="outt")
            nc.scalar.copy(outt[:, 0:NIN], dwin_ps[:])
            nc.scalar.copy(outt[:, NIN:NIN + NH], dwA_ps[:, 0:NH])
            nc.vector.tensor_copy(outt[:, NIN + NH:NIN + N], dwB_ps[:])
            nc.vector.tensor_copy(outt[:, NIN + N:OUT_W], dwA_ps[:, NH:NH + K])
            nc.sync.dma_start(out=out_d.ap(), in_=outt[:])

    nc.compile()
    return nc


_NC_CACHE = None


def _get_nc():
    global _NC_CACHE
    if _NC_CACHE is None:
        _NC_CACHE = _build_program()
    return _NC_CACHE


def _prep_core_inputs(v, z, x, error1, error2, w_out, b, h):
    jsl = slice(h * NH, (h + 1) * NH)
    osl = slice((1 - h) * NH, (2 - h) * NH)
    rev = slice(None, None, -1)

    z_r = z[b, rev, :]                         # [T, N]
    x_r = x[b, rev, :]                         # [T, NIN]
    zsh = np.vstack([z_r[1:], np.zeros((1, N), np.float32)])

    feat = np.zeros((NH, FEAT_W), np.float32)
    feat[:, 0:T] = v[b, rev, jsl].T
    feat[:, T:2 * T] = z_r[:, jsl].T           # cols 600:604 stay zero
    feat[:, FEAT_W - 1] = (REG / (B * T)) * error2[jsl]

    lhs = np.empty((TC, Q, LHS_W), np.float32)
    for q in range(Q):
        rows = slice(q * TC, (q + 1) * TC)
        lhs[:, q, 0:NIN] = x_r[rows]
        lhs[:, q, NIN:NIN + NH] = zsh[rows][:, jsl]
        lhs[:, q, NIN + NH:LHS_W] = zsh[rows][:, osl]

    small = np.zeros((K, SMALL_W), np.float32)
    small[:, 0:T] = error1[b, rev, :].T            # col T stays zero (scan pad)
    small[:, T + 1:T + 1 + NH] = w_out[jsl, :].T
    small[0, T + 1 + NH:T + 1 + 2 * NH] = z_r[0, jsl]      # z_u[0] (own half)
    small[0, T + 1 + 2 * NH:T + 1 + 2 * NH + K] = error1[b, T - 1, :]  # e1f[0]

    return {"feat": feat, "lhsT": np.ascontiguousarray(lhs), "small": small,
            "identd": np.eye(NH, dtype=np.float32)}


def kernel(v, z, x, error1, error2, w_out, _trace=False):
    v = np.asarray(v, np.float32)
    z = np.asarray(z, np.float32)
    x = np.asarray(x, np.float32)
    error1 = np.asarray(error1, np.float32)
    error2 = np.asarray(error2, np.float32)
    w_out = np.asarray(w_out, np.float32)

    nc = _get_nc()
    in_maps = [_prep_core_inputs(v, z, x, error1, error2, w_out, c // 2, c % 2)
               for c in range(N_CORES)]
    res = run_bass_kernel_spmd(nc, in_maps, core_ids=list(range(N_CORES)),
                               trace=_trace)

    dw_in = np.zeros((NIN, N), np.float32)
    dw_rec = np.zeros((N, N), np.float32)
    dw_out = np.zeros((N, K), np.float32)
    for h in range(2):
        jsl = slice(h * NH, (h + 1) * NH)
        osl = slice((1 - h) * NH, (2 - h) * NH)
        s = np.zeros((NH, OUT_W), np.float64)
        for b in range(B):
            s += res.results[2 * b + h]["out"]
        dw_in[:, jsl] = 0.5 * s[:, 0:NIN]
        dw_rec[jsl, jsl] = 0.5 * s[:, NIN:NIN + NH]      # own-half rows
        dw_rec[osl, jsl] = 0.5 * s[:, NIN + NH:NIN + N]  # other-half rows
        dw_out[jsl, :] = s[:, NIN + N:OUT_W]
    np.fill_diagonal(dw_rec, 0.0)

    if _trace:
        return (dw_in, dw_rec, dw_out), res
    return dw_in, dw_rec, dw_out
